# revision 17
# baseline (speedup 1.0000x reference)
"""Distributed single-head attention for TRN2 (8 NeuronCores).

Reference computation (per batch b):
    q = x @ Wq; k = x @ Wk; v = x @ Wv          (x: [S, E])
    s = (q @ k.T) / sqrt(DK) - 1e15 * mask
    out = softmax(s, axis=-1) @ v               ([S, DV])

Sharding: 8 cores = 4 batches x 2 sequence halves. Each core computes
attention for 1024 queries of one batch; K/V are recomputed per core from
the full sequence (cheap vs. the attention matmuls, avoids collectives).

Host-prepared layouts (host pre/post-processing is free):
  - xt  [E, S]  bf16: x_b^T, sequence permuted so this core's query half
                occupies columns [0, 1024). Permutation-invariant softmax.
  - wq  [E, DK] bf16: Wq pre-scaled by 1/sqrt(DK).
  - mc  [S, SQ] bf16: (1 - mask) transposed to [key, query], keys
                permuted like xt. exp(s - 1e15*m) == exp(s) * (1 - m).
  - out_ot  [DV, SQ] bf16: UNNORMALIZED numerator in [dv, q] layout.
  - out_acc [P, SQ] bf16: per-key-lane partials of masked probabilities;
                rowsum[q] = sum_p out_acc[p, q]. Softmax division done on
                the host; removes the reciprocal/transpose epilogue.

Device schedule (PE-bound throughout; ACT exp ~1.12us/tile and DVE
mask-mult + acc-add ~1.37us/tile hide under PE work):
  - All input DMAs on the sync HWDGE ring, FIFO: weights + x chunks at
    full HBM bandwidth first, then the mask in 4 grouped transfers.
  - Warmup matmuls on zeros during the initial DMA wait (HAM un-throttle).
  - Q + V projections interleaved per x-chunk (6 PSUM banks), then the
    whole VT->V [k, dv] layout change as ONE batched dma_start_transpose
    ([128, 16, 128] destination) on the scalar ring.
  - K projection split in two key-halves (separate kt tiles so tile
    granularity dependencies don't serialize): half 0 before the
    attention loop; half 1 interleaved 2-matmuls-per-tile into attention
    tiles 0..7, which overlaps the DVE/ACT-heavy early attention with
    PE-heavy projection work.
  - Attention tile t: ST[k128,q] = KT_t^T QT (2 MM) -> P = exp(ST) (ACT)
    -> P *= mc (DVE) -> acc += P (DVE) -> OT += V_t^T P (2 MM, deferred
    two tiles so V transpose latency never stalls the PE queue).
"""

import math
from contextlib import ExitStack

import ml_dtypes
import numpy as np

import concourse.bass as bass
import concourse.tile as tile
from concourse import bacc, mybir
from concourse.bass_utils import run_bass_kernel_spmd

B, S, E, DK, DV = 4, 2048, 1024, 128, 128
SQ = S // 2  # queries per core
P = 128  # SBUF partitions
EC = E // P  # contraction chunks for projections
KTILES = S // P  # key tiles
MG = 4  # mask DMA groups (4 key tiles each)
AV_LAG = 2  # AV matmuls trail scores by this many tiles

f32 = mybir.dt.float32
bf16 = mybir.dt.bfloat16

# test.py pokes these to get profiling info
TRACE = False
LAST_RESULT = None

N_WARMUP_MM = 9  # dummy matmuls to warm the PE HAM clock during DMA wait


def build():
    nc = bacc.Bacc()
    xt = nc.declare_dram_parameter("xt", [E, S], bf16, isOutput=False)
    # weights arrive host-packed as [p, c*DK+d] = W[c*128+p, d] so the load
    # is one fully-contiguous DMA (2KB/partition descriptors)
    wq = nc.declare_dram_parameter("wq", [P, EC * DK], bf16, isOutput=False)
    wk = nc.declare_dram_parameter("wk", [P, EC * DK], bf16, isOutput=False)
    wv = nc.declare_dram_parameter("wv", [P, EC * DV], bf16, isOutput=False)
    mc = nc.declare_dram_parameter("mc", [S, SQ], bf16, isOutput=False)
    out_ot = nc.declare_dram_parameter("out_ot", [DV, SQ], bf16, isOutput=True)
    out_acc = nc.declare_dram_parameter("out_acc", [P, SQ], bf16, isOutput=True)

    with ExitStack() as ctx:
        tc = ctx.enter_context(tile.TileContext(nc))
        const_pool = ctx.enter_context(tc.tile_pool(name="const", bufs=1))
        in_pool = ctx.enter_context(tc.tile_pool(name="inputs", bufs=1))
        proj_sb = ctx.enter_context(tc.tile_pool(name="proj", bufs=1))
        p_pool = ctx.enter_context(tc.tile_pool(name="p", bufs=4))
        stat = ctx.enter_context(tc.tile_pool(name="stat", bufs=1))
        # kt1 accumulates during the attention loop, so it outlives proj_ps;
        # pools release in stack order, so it must be created first
        kt1_pool = ctx.enter_context(tc.tile_pool(name="kt1_ps", bufs=2, space="PSUM"))
        proj_ctx = ctx.enter_context(ExitStack())
        proj_ps = proj_ctx.enter_context(
            tc.tile_pool(name="proj_ps", bufs=6, space="PSUM")
        )

        zeros_w = const_pool.tile([P, 512], bf16)
        nc.vector.memset(zeros_w[:], 0.0)
        acc = stat.tile([P, SQ], bf16)
        nc.vector.memset(acc[:], 0.0)

        # --- input DMAs, all on the sync HWDGE ring: FIFO order gives the
        # projection-critical tensors full HBM bandwidth before the masks.
        w_sb = {}
        for name, w in (("wq", wq), ("wk", wk), ("wv", wv)):
            wt = in_pool.tile([P, EC * DK], bf16, tag=name)
            w_sb[name] = wt

        x_sb = []
        for c in range(EC):
            xc = in_pool.tile([P, S], bf16, tag=f"x{c}")
            x_sb.append(xc)

        # mask in MG grouped tiles [128, KTILES//MG, SQ]:
        # m_sb[g][p, i, q] = mc[(g*KTILES//MG + i)*128 + p, q]
        TPG = KTILES // MG  # key tiles per mask group
        m_sb = []
        for g in range(MG):
            mtile = in_pool.tile([P, TPG, SQ], bf16, tag=f"m{g}")
            m_sb.append(mtile)

        nc.sync.dma_start(w_sb["wq"][:], wq[:, :])
        nc.sync.dma_start(x_sb[0][:], xt[0:P, :])
        nc.sync.dma_start(w_sb["wk"][:], wk[:, :])
        nc.sync.dma_start(w_sb["wv"][:], wv[:, :])
        for c in range(1, EC):
            nc.sync.dma_start(x_sb[c][:], xt[c * P : (c + 1) * P, :])
        mc_v = mc.rearrange("(g i p) q -> g p i q", g=MG, i=TPG, p=P)
        for g in range(MG):
            nc.sync.dma_start(m_sb[g][:, :, :], mc_v[g])

        # --- PE warmup: dummy matmuls on zeros while the first DMAs land,
        # so the HAM clock-gate un-throttles (1.2 -> 2.4 GHz) before the
        # real projection matmuls start.
        warm_ps = proj_ps.tile([P, 512], f32, tag="pps")
        for _ in range(N_WARMUP_MM):
            nc.tensor.matmul(
                warm_ps[:], zeros_w[:, 0:P], zeros_w[:], start=True, stop=True
            )

        # --- Q + V projections interleaved per x-chunk: QT [d, q] and
        # VT [d, k] in four 512-col quarters (2 + 4 PSUM banks).
        qt_sb = proj_sb.tile([P, SQ], bf16)
        vt_sb = proj_sb.tile([P, S], bf16)
        v_sb = proj_sb.tile([P, KTILES, DV], bf16)

        def alloc_ps(n, tag="pps"):
            pss = []
            for _ in range(n):
                ps = proj_ps.tile([P, 512], f32, tag=tag)
                pss.append(ps)
            return pss

        qt_ps = alloc_ps(2)
        vt_ps = alloc_ps(4)
        for c in range(EC):
            for j in range(2):
                nc.tensor.matmul(
                    qt_ps[j][:],
                    w_sb["wq"][:, c * DK : (c + 1) * DK],
                    x_sb[c][:, j * 512 : (j + 1) * 512],
                    start=(c == 0),
                    stop=(c == EC - 1),
                )
            for g in range(4):
                nc.tensor.matmul(
                    vt_ps[g][:],
                    w_sb["wv"][:, c * DV : (c + 1) * DV],
                    x_sb[c][:, g * 512 : (g + 1) * 512],
                    start=(c == 0),
                    stop=(c == EC - 1),
                )
        # qt j0 on ACT (early, cheap); everything V on DVE so the scalar
        # queue stays clear for the V transpose issue + exps.
        nc.scalar.copy(qt_sb[:, 0:512], qt_ps[0][:])
        nc.vector.tensor_copy(qt_sb[:, 512:1024], qt_ps[1][:])
        for g in range(4):
            nc.vector.tensor_copy(vt_sb[:, g * 512 : (g + 1) * 512], vt_ps[g][:])
        # ONE batched transpose: v_sb[k, t, dv] = vt_sb[dv, t*128+k]
        nc.scalar.dma_start_transpose(v_sb[:, :, :], vt_sb[:, :])

        # --- K projection, half 0 (key tiles 0..7) ---
        kt0_sb = proj_sb.tile([P, SQ], bf16)
        kt1_sb = proj_sb.tile([P, SQ], bf16)
        kt0_ps = alloc_ps(2)
        for c in range(EC):
            for j in range(2):
                nc.tensor.matmul(
                    kt0_ps[j][:],
                    w_sb["wk"][:, c * DK : (c + 1) * DK],
                    x_sb[c][:, j * 512 : (j + 1) * 512],
                    start=(c == 0),
                    stop=(c == EC - 1),
                )
        nc.scalar.copy(kt0_sb[:, 0:512], kt0_ps[0][:])
        nc.vector.tensor_copy(kt0_sb[:, 512:1024], kt0_ps[1][:])
        kt1_ps = []
        for _ in range(2):
            kp = kt1_pool.tile([P, 512], f32, tag="kt1")
            kt1_ps.append(kp)
        proj_ctx.close()  # free the 6 projection PSUM banks for st/ot

        st_pool = ctx.enter_context(tc.tile_pool(name="st_ps", bufs=2, space="PSUM"))
        ot_pool = ctx.enter_context(tc.tile_pool(name="ot_ps", bufs=1, space="PSUM"))
        ot = ot_pool.tile([P, SQ], f32)  # OT [dv, q] accumulator

        # --- attention over key tiles; K half 1 (key tiles 8..15) is
        # interleaved 2 matmuls per tile into tiles 0..7, and AV trails
        # scores by AV_LAG tiles so the V transpose never stalls PE ---
        p_tiles = [None] * KTILES

        def kt_slice(t):
            src = kt0_sb if t < 8 else kt1_sb
            return src[:, (t % 8) * P : (t % 8 + 1) * P]

        def emit_scores(t):
            st = st_pool.tile([P, SQ], f32, tag="st")
            for j in range(2):
                nc.tensor.matmul(
                    st[:, j * 512 : (j + 1) * 512],
                    kt_slice(t),
                    qt_sb[:, j * 512 : (j + 1) * 512],
                    start=True,
                    stop=True,
                )
            p = p_pool.tile([P, SQ], bf16, tag="p")
            p_tiles[t] = p
            nc.scalar.activation(p[:], st[:], mybir.ActivationFunctionType.Exp)
            # zero the masked entries: exp(s - 1e15*m) == exp(s) * (1 - m)
            nc.vector.tensor_mul(p[:], p[:], m_sb[t // (KTILES // MG)][:, t % (KTILES // MG), :])
            # acc += p runs one tile late so AV_t never waits behind it on
            # DVE; the last three adds move past the loop entirely so the
            # final AV matmuls only ever wait on their own mask-multiply
            if 0 < t <= KTILES - 3:
                nc.vector.tensor_add(acc[:], acc[:], p_tiles[t - 1][:])

        def emit_av(t):
            p = p_tiles[t]
            for j in range(2):
                nc.tensor.matmul(
                    ot[:, j * 512 : (j + 1) * 512],
                    v_sb[:, t, :],
                    p[:, j * 512 : (j + 1) * 512],
                    start=(t == 0),
                    stop=(t == KTILES - 1),
                )

        for t in range(KTILES):
            emit_scores(t)
            if t < 4:
                # four K-half-1 matmuls per tile: (j, c) pairs in c-major
                # order so each PSUM group accumulates chunks in sequence
                for i in range(4):
                    idx = 4 * t + i
                    j, c = idx // 8, idx % 8
                    nc.tensor.matmul(
                        kt1_ps[j][:],
                        w_sb["wk"][:, c * DK : (c + 1) * DK],
                        x_sb[c][:, 1024 + j * 512 : 1024 + (j + 1) * 512],
                        start=(c == 0),
                        stop=(c == EC - 1),
                    )
                if t == 3:
                    nc.scalar.copy(kt1_sb[:, 0:512], kt1_ps[0][:])
                    nc.vector.tensor_copy(kt1_sb[:, 512:1024], kt1_ps[1][:])
            if t >= AV_LAG:
                emit_av(t - AV_LAG)
        for t in range(KTILES - AV_LAG, KTILES):
            emit_av(t)
        for t in range(KTILES - 3, KTILES):
            nc.vector.tensor_add(acc[:], acc[:], p_tiles[t][:])

        # --- epilogue: ship OT (unnormalized) and acc; host normalizes ---
        o_sb = stat.tile([P, SQ], bf16)
        nc.scalar.copy(o_sb[:, 0:512], ot[:, 0:512])
        nc.vector.tensor_copy(o_sb[:, 512:1024], ot[:, 512:1024])
        nc.sync.dma_start(out_ot[:, :], o_sb[:])
        nc.scalar.dma_start(out_acc[:, :], acc[:])

    nc.compile()
    return nc


_NC_CACHE = None


def kernel(inputs, mask, Wq, Wk, Wv):
    global _NC_CACHE, LAST_RESULT
    inputs = np.asarray(inputs)
    mask = np.asarray(mask)
    bf = ml_dtypes.bfloat16
    scale = np.float32(1.0 / math.sqrt(DK))

    def pack_w(w):  # [E, DK] -> [p, c*DK+d] = w[c*128+p, d]
        w = np.asarray(w).astype(bf)
        return np.ascontiguousarray(
            w.reshape(EC, P, DK).transpose(1, 0, 2).reshape(P, EC * DK)
        )

    wq_h = pack_w(np.asarray(Wq) * scale)
    wk_h = pack_w(Wk)
    wv_h = pack_w(Wv)

    if _NC_CACHE is None:
        _NC_CACHE = build()
    nc = _NC_CACHE

    in_maps = []
    for core in range(8):
        b, h = divmod(core, 2)
        q0 = h * SQ
        idx = np.r_[q0:S, 0:q0]  # rotate so this core's queries come first
        xb = inputs[b]  # [S, E] f32
        xt_core = np.ascontiguousarray(xb[idx].T).astype(bf)  # [E, S]
        mc_core = np.ascontiguousarray(
            (1 - mask[b, q0 : q0 + SQ, :][:, idx]).T.astype(np.float32)
        ).astype(bf)  # [S, SQ] complement, [key, query]
        in_maps.append(
            {"xt": xt_core, "wq": wq_h, "wk": wk_h, "wv": wv_h, "mc": mc_core}
        )

    res = run_bass_kernel_spmd(nc, in_maps, list(range(8)), trace=TRACE)
    LAST_RESULT = res
    outp = np.empty((B, S, DV), np.float32)
    for core in range(8):
        b, h = divmod(core, 2)
        q0 = h * SQ
        ot = np.asarray(res.results[core]["out_ot"]).astype(np.float32)  # [DV, SQ]
        acc = np.asarray(res.results[core]["out_acc"]).astype(np.float32)  # [P, SQ]
        rowsum = acc.sum(axis=0)  # [SQ]
        outp[b, q0 : q0 + SQ, :] = (ot / rowsum[None, :]).T
    return outp


# revision 18
# speedup vs baseline: 1.1279x; 1.1279x over previous
"""Distributed single-head attention for TRN2 (8 NeuronCores).

Reference computation (per batch b):
    q = x @ Wq; k = x @ Wk; v = x @ Wv          (x: [S, E])
    s = (q @ k.T) / sqrt(DK) - 1e15 * mask
    out = softmax(s, axis=-1) @ v               ([S, DV])

Sharding: 8 cores = 4 batches x 2 sequence halves. Each core computes
attention for 1024 queries of one batch; K/V are recomputed per core from
the full sequence (cheap vs. the attention matmuls, avoids collectives).

Host-prepared layouts (host pre/post-processing is free):
  - xt  [E, S]  bf16: x_b^T, sequence permuted so this core's query half
                occupies columns [0, 1024). Permutation-invariant softmax.
  - wq  [E, DK] bf16: Wq pre-scaled by 1/sqrt(DK).
  - mc  [S, SQ] bf16: (1 - mask) transposed to [key, query], keys
                permuted like xt. exp(s - 1e15*m) == exp(s) * (1 - m).
  - out_ot  [DV, SQ] bf16: UNNORMALIZED numerator in [dv, q] layout.
  - out_acc [P, SQ] bf16: per-key-lane partials of masked probabilities;
                rowsum[q] = sum_p out_acc[p, q]. Softmax division done on
                the host; removes the reciprocal/transpose epilogue.

Device schedule (PE-bound throughout; ACT exp ~1.12us/tile and DVE
mask-mult + acc-add ~1.37us/tile hide under PE work):
  - All input DMAs on the sync HWDGE ring, FIFO: weights + x chunks at
    full HBM bandwidth first, then the mask in 4 grouped transfers.
  - Warmup matmuls on zeros during the initial DMA wait (HAM un-throttle).
  - Q + V projections interleaved per x-chunk (6 PSUM banks), then the
    whole VT->V [k, dv] layout change as ONE batched dma_start_transpose
    ([128, 16, 128] destination) on the scalar ring.
  - K projection split in two key-halves (separate kt tiles so tile
    granularity dependencies don't serialize): half 0 before the
    attention loop; half 1 interleaved 2-matmuls-per-tile into attention
    tiles 0..7, which overlaps the DVE/ACT-heavy early attention with
    PE-heavy projection work.
  - Attention tile t: ST[k128,q] = KT_t^T QT (2 MM) -> P = exp(ST) (ACT)
    -> P *= mc (DVE) -> acc += P (DVE) -> OT += V_t^T P (2 MM, deferred
    two tiles so V transpose latency never stalls the PE queue).
"""

import math
from contextlib import ExitStack

import ml_dtypes
import numpy as np

import concourse.bass as bass
import concourse.tile as tile
from concourse import bacc, mybir
from concourse.bass_utils import run_bass_kernel_spmd

B, S, E, DK, DV = 4, 2048, 1024, 128, 128
SQ = S // 2  # queries per core
P = 128  # SBUF partitions
EC = E // P  # contraction chunks for projections
KTILES = S // P  # key tiles
MG = 4  # mask DMA groups (4 key tiles each)
AV_LAG = 3  # AV matmuls trail scores by this many tiles

f32 = mybir.dt.float32
bf16 = mybir.dt.bfloat16

# test.py pokes these to get profiling info
TRACE = False
LAST_RESULT = None

N_WARMUP_MM = 9  # dummy matmuls to warm the PE HAM clock during DMA wait


def build():
    nc = bacc.Bacc()
    xt = nc.declare_dram_parameter("xt", [E, S], bf16, isOutput=False)
    # weights arrive host-packed as [p, c*DK+d] = W[c*128+p, d] so the load
    # is one fully-contiguous DMA (2KB/partition descriptors)
    wq = nc.declare_dram_parameter("wq", [P, EC * DK], bf16, isOutput=False)
    wk = nc.declare_dram_parameter("wk", [P, EC * DK], bf16, isOutput=False)
    wv = nc.declare_dram_parameter("wv", [P, EC * DV], bf16, isOutput=False)
    mc = nc.declare_dram_parameter("mc", [S, SQ], bf16, isOutput=False)
    out_ot = nc.declare_dram_parameter("out_ot", [DV, SQ], bf16, isOutput=True)
    out_acc = nc.declare_dram_parameter("out_acc", [P, SQ], bf16, isOutput=True)

    with ExitStack() as ctx:
        tc = ctx.enter_context(tile.TileContext(nc))
        const_pool = ctx.enter_context(tc.tile_pool(name="const", bufs=1))
        in_pool = ctx.enter_context(tc.tile_pool(name="inputs", bufs=1))
        proj_sb = ctx.enter_context(tc.tile_pool(name="proj", bufs=1))
        p_pool = ctx.enter_context(tc.tile_pool(name="p", bufs=6))
        stat = ctx.enter_context(tc.tile_pool(name="stat", bufs=1))
        proj_ctx = ctx.enter_context(ExitStack())
        proj_ps = proj_ctx.enter_context(
            tc.tile_pool(name="proj_ps", bufs=8, space="PSUM")
        )

        zeros_w = const_pool.tile([P, 512], bf16)
        nc.vector.memset(zeros_w[:], 0.0)
        acc = stat.tile([P, SQ], bf16)
        nc.vector.memset(acc[:], 0.0)

        # --- input DMAs, all on the sync HWDGE ring: FIFO order gives the
        # projection-critical tensors full HBM bandwidth before the masks.
        w_sb = {}
        for name, w in (("wq", wq), ("wk", wk), ("wv", wv)):
            wt = in_pool.tile([P, EC * DK], bf16, tag=name)
            w_sb[name] = wt

        x_sb = []
        for c in range(EC):
            xc = in_pool.tile([P, S], bf16, tag=f"x{c}")
            x_sb.append(xc)

        # mask in MG grouped tiles [128, KTILES//MG, SQ]:
        # m_sb[g][p, i, q] = mc[(g*KTILES//MG + i)*128 + p, q]
        TPG = KTILES // MG  # key tiles per mask group
        m_sb = []
        for g in range(MG):
            mtile = in_pool.tile([P, TPG, SQ], bf16, tag=f"m{g}")
            m_sb.append(mtile)

        nc.sync.dma_start(w_sb["wq"][:], wq[:, :])
        nc.sync.dma_start(x_sb[0][:], xt[0:P, :])
        nc.sync.dma_start(w_sb["wk"][:], wk[:, :])
        nc.sync.dma_start(w_sb["wv"][:], wv[:, :])
        for c in range(1, EC):
            nc.sync.dma_start(x_sb[c][:], xt[c * P : (c + 1) * P, :])
        mc_v = mc.rearrange("(g i p) q -> g p i q", g=MG, i=TPG, p=P)
        for g in range(MG):
            nc.sync.dma_start(m_sb[g][:, :, :], mc_v[g])

        # --- PE warmup: dummy matmuls on zeros while the first DMAs land,
        # so the HAM clock-gate un-throttles (1.2 -> 2.4 GHz) before the
        # real projection matmuls start.
        warm_ps = proj_ps.tile([P, 512], f32, tag="pps")
        for _ in range(N_WARMUP_MM):
            nc.tensor.matmul(
                warm_ps[:], zeros_w[:, 0:P], zeros_w[:], start=True, stop=True
            )

        # --- Q + V projections interleaved per x-chunk: QT [d, q] and
        # VT [d, k] in four 512-col quarters (2 + 4 PSUM banks).
        qt_sb = proj_sb.tile([P, SQ], bf16)
        vt_sb = proj_sb.tile([P, S], bf16)
        v_sb = proj_sb.tile([P, KTILES, DV], bf16)

        def alloc_ps(n, tag="pps"):
            pss = []
            for _ in range(n):
                ps = proj_ps.tile([P, 512], f32, tag=tag)
                pss.append(ps)
            return pss

        qt_ps = alloc_ps(2)
        vt_ps = alloc_ps(4)
        kt0_ps = alloc_ps(2)
        for c in range(EC):
            for j in range(2):
                nc.tensor.matmul(
                    qt_ps[j][:],
                    w_sb["wq"][:, c * DK : (c + 1) * DK],
                    x_sb[c][:, j * 512 : (j + 1) * 512],
                    start=(c == 0),
                    stop=(c == EC - 1),
                )
            for g in range(4):
                nc.tensor.matmul(
                    vt_ps[g][:],
                    w_sb["wv"][:, c * DV : (c + 1) * DV],
                    x_sb[c][:, g * 512 : (g + 1) * 512],
                    start=(c == 0),
                    stop=(c == EC - 1),
                )
            for j in range(2):
                nc.tensor.matmul(
                    kt0_ps[j][:],
                    w_sb["wk"][:, c * DK : (c + 1) * DK],
                    x_sb[c][:, j * 512 : (j + 1) * 512],
                    start=(c == 0),
                    stop=(c == EC - 1),
                )
        # qt j0 on ACT (early, cheap); everything V on DVE so the scalar
        # queue stays clear for the V transpose issue + exps.
        nc.scalar.copy(qt_sb[:, 0:512], qt_ps[0][:])
        nc.vector.tensor_copy(qt_sb[:, 512:1024], qt_ps[1][:])
        for g in range(4):
            nc.vector.tensor_copy(vt_sb[:, g * 512 : (g + 1) * 512], vt_ps[g][:])
        # ONE batched transpose: v_sb[k, t, dv] = vt_sb[dv, t*128+k]
        nc.scalar.dma_start_transpose(v_sb[:, :, :], vt_sb[:, :])

        # --- K half-0 copies (matmuls ran inside the chunk loop above) ---
        kt0_sb = proj_sb.tile([P, SQ], bf16)
        kt1_sb = proj_sb.tile([P, SQ], bf16)
        nc.scalar.copy(kt0_sb[:, 0:512], kt0_ps[0][:])
        nc.vector.tensor_copy(kt0_sb[:, 512:1024], kt0_ps[1][:])
        proj_ctx.close()  # free all 8 projection PSUM banks
        # kt1 accumulates during attention tiles 0..3 (2 banks), st double-
        # buffered (4 banks), ot accumulator (2 banks): exactly 8
        kt1_pool = ctx.enter_context(tc.tile_pool(name="kt1_ps", bufs=2, space="PSUM"))
        kt1_ps = []
        for _ in range(2):
            kp = kt1_pool.tile([P, 512], f32, tag="kt1")
            kt1_ps.append(kp)

        st_pool = ctx.enter_context(tc.tile_pool(name="st_ps", bufs=2, space="PSUM"))
        ot_pool = ctx.enter_context(tc.tile_pool(name="ot_ps", bufs=1, space="PSUM"))
        ot = ot_pool.tile([P, SQ], f32)  # OT [dv, q] accumulator

        # --- attention over key tiles; K half 1 (key tiles 8..15) is
        # interleaved 2 matmuls per tile into tiles 0..7, and AV trails
        # scores by AV_LAG tiles so the V transpose never stalls PE ---
        p_tiles = [None] * KTILES

        def kt_slice(t):
            src = kt0_sb if t < 8 else kt1_sb
            return src[:, (t % 8) * P : (t % 8 + 1) * P]

        def emit_scores(t):
            st = st_pool.tile([P, SQ], f32, tag="st")
            for j in range(2):
                nc.tensor.matmul(
                    st[:, j * 512 : (j + 1) * 512],
                    kt_slice(t),
                    qt_sb[:, j * 512 : (j + 1) * 512],
                    start=True,
                    stop=True,
                )
            p = p_pool.tile([P, SQ], bf16, tag="p")
            p_tiles[t] = p
            nc.scalar.activation(p[:], st[:], mybir.ActivationFunctionType.Exp)
            # zero the masked entries: exp(s - 1e15*m) == exp(s) * (1 - m)
            nc.vector.tensor_mul(p[:], p[:], m_sb[t // (KTILES // MG)][:, t % (KTILES // MG), :])
            # acc += p runs one tile late so AV_t never waits behind it on
            # DVE; the last three adds move past the loop entirely so the
            # final AV matmuls only ever wait on their own mask-multiply
            if 0 < t <= KTILES - 3:
                nc.vector.tensor_add(acc[:], acc[:], p_tiles[t - 1][:])

        def emit_av(t):
            p = p_tiles[t]
            for j in range(2):
                nc.tensor.matmul(
                    ot[:, j * 512 : (j + 1) * 512],
                    v_sb[:, t, :],
                    p[:, j * 512 : (j + 1) * 512],
                    start=(t == 0),
                    stop=(t == KTILES - 1),
                )

        for t in range(KTILES):
            emit_scores(t)
            if t < 4:
                # four K-half-1 matmuls per tile: (j, c) pairs in c-major
                # order so each PSUM group accumulates chunks in sequence
                for i in range(4):
                    idx = 4 * t + i
                    j, c = idx // 8, idx % 8
                    nc.tensor.matmul(
                        kt1_ps[j][:],
                        w_sb["wk"][:, c * DK : (c + 1) * DK],
                        x_sb[c][:, 1024 + j * 512 : 1024 + (j + 1) * 512],
                        start=(c == 0),
                        stop=(c == EC - 1),
                    )
                if t == 3:
                    nc.scalar.copy(kt1_sb[:, 0:512], kt1_ps[0][:])
                    nc.vector.tensor_copy(kt1_sb[:, 512:1024], kt1_ps[1][:])
            if t >= AV_LAG:
                emit_av(t - AV_LAG)
        for t in range(KTILES - AV_LAG, KTILES):
            emit_av(t)
        for t in range(KTILES - 3, KTILES):
            nc.vector.tensor_add(acc[:], acc[:], p_tiles[t][:])

        # --- epilogue: ship OT (unnormalized) and acc; host normalizes ---
        o_sb = stat.tile([P, SQ], bf16)
        nc.scalar.copy(o_sb[:, 0:512], ot[:, 0:512])
        nc.vector.tensor_copy(o_sb[:, 512:1024], ot[:, 512:1024])
        nc.sync.dma_start(out_ot[:, :], o_sb[:])
        nc.scalar.dma_start(out_acc[:, :], acc[:])

    nc.compile()
    return nc


_NC_CACHE = None


def kernel(inputs, mask, Wq, Wk, Wv):
    global _NC_CACHE, LAST_RESULT
    inputs = np.asarray(inputs)
    mask = np.asarray(mask)
    bf = ml_dtypes.bfloat16
    scale = np.float32(1.0 / math.sqrt(DK))

    def pack_w(w):  # [E, DK] -> [p, c*DK+d] = w[c*128+p, d]
        w = np.asarray(w).astype(bf)
        return np.ascontiguousarray(
            w.reshape(EC, P, DK).transpose(1, 0, 2).reshape(P, EC * DK)
        )

    wq_h = pack_w(np.asarray(Wq) * scale)
    wk_h = pack_w(Wk)
    wv_h = pack_w(Wv)

    if _NC_CACHE is None:
        _NC_CACHE = build()
    nc = _NC_CACHE

    in_maps = []
    for core in range(8):
        b, h = divmod(core, 2)
        q0 = h * SQ
        idx = np.r_[q0:S, 0:q0]  # rotate so this core's queries come first
        xb = inputs[b]  # [S, E] f32
        xt_core = np.ascontiguousarray(xb[idx].T).astype(bf)  # [E, S]
        mc_core = np.ascontiguousarray(
            (1 - mask[b, q0 : q0 + SQ, :][:, idx]).T.astype(np.float32)
        ).astype(bf)  # [S, SQ] complement, [key, query]
        in_maps.append(
            {"xt": xt_core, "wq": wq_h, "wk": wk_h, "wv": wv_h, "mc": mc_core}
        )

    res = run_bass_kernel_spmd(nc, in_maps, list(range(8)), trace=TRACE)
    LAST_RESULT = res
    outp = np.empty((B, S, DV), np.float32)
    for core in range(8):
        b, h = divmod(core, 2)
        q0 = h * SQ
        ot = np.asarray(res.results[core]["out_ot"]).astype(np.float32)  # [DV, SQ]
        acc = np.asarray(res.results[core]["out_acc"]).astype(np.float32)  # [P, SQ]
        rowsum = acc.sum(axis=0)  # [SQ]
        outp[b, q0 : q0 + SQ, :] = (ot / rowsum[None, :]).T
    return outp


# revision 19
# speedup vs baseline: 1.1545x; 1.0235x over previous
"""Distributed single-head attention for TRN2 (8 NeuronCores).

Reference computation (per batch b):
    q = x @ Wq; k = x @ Wk; v = x @ Wv          (x: [S, E])
    s = (q @ k.T) / sqrt(DK) - 1e15 * mask
    out = softmax(s, axis=-1) @ v               ([S, DV])

Sharding: 8 cores = 4 batches x 2 sequence halves. Each core computes
attention for 1024 queries of one batch; K/V are recomputed per core from
the full sequence (cheap vs. the attention matmuls, avoids collectives).

Host-prepared layouts (host pre/post-processing is free):
  - xt  [E, S]  bf16: x_b^T, sequence permuted so this core's query half
                occupies columns [0, 1024). Permutation-invariant softmax.
  - wq  [E, DK] bf16: Wq pre-scaled by 1/sqrt(DK).
  - mc  [S, SQ] bf16: (1 - mask) transposed to [key, query], keys
                permuted like xt. exp(s - 1e15*m) == exp(s) * (1 - m).
  - out_ot  [DV, SQ] bf16: UNNORMALIZED numerator in [dv, q] layout.
  - out_acc [P, SQ] bf16: per-key-lane partials of masked probabilities;
                rowsum[q] = sum_p out_acc[p, q]. Softmax division done on
                the host; removes the reciprocal/transpose epilogue.

Device schedule (PE-bound throughout; ACT exp ~1.12us/tile and DVE
mask-mult + acc-add ~1.37us/tile hide under PE work):
  - All input DMAs on the sync HWDGE ring, FIFO: weights + x chunks at
    full HBM bandwidth first, then the mask in 4 grouped transfers.
  - Warmup matmuls on zeros during the initial DMA wait (HAM un-throttle).
  - Q + V projections interleaved per x-chunk (6 PSUM banks), then the
    whole VT->V [k, dv] layout change as ONE batched dma_start_transpose
    ([128, 16, 128] destination) on the scalar ring.
  - K projection split in two key-halves (separate kt tiles so tile
    granularity dependencies don't serialize): half 0 before the
    attention loop; half 1 interleaved 2-matmuls-per-tile into attention
    tiles 0..7, which overlaps the DVE/ACT-heavy early attention with
    PE-heavy projection work.
  - Attention tile t: ST[k128,q] = KT_t^T QT (2 MM) -> P = exp(ST) (ACT)
    -> P *= mc (DVE) -> acc += P (DVE) -> OT += V_t^T P (2 MM, deferred
    two tiles so V transpose latency never stalls the PE queue).
"""

import math
from contextlib import ExitStack

import ml_dtypes
import numpy as np

import concourse.bass as bass
import concourse.tile as tile
from concourse import bacc, mybir
from concourse.bass_utils import run_bass_kernel_spmd

B, S, E, DK, DV = 4, 2048, 1024, 128, 128
SQ = S // 2  # queries per core
P = 128  # SBUF partitions
EC = E // P  # contraction chunks for projections
KTILES = S // P  # key tiles
MG = 4  # mask DMA groups (4 key tiles each)
AV_LAG = 3  # AV matmuls trail scores by this many tiles

f32 = mybir.dt.float32
bf16 = mybir.dt.bfloat16

# test.py pokes these to get profiling info
TRACE = False
LAST_RESULT = None

N_WARMUP_MM = 9  # dummy matmuls to warm the PE HAM clock during DMA wait


def build():
    nc = bacc.Bacc()
    xt = nc.declare_dram_parameter("xt", [E, S], bf16, isOutput=False)
    # weights arrive host-packed as [p, c*DK+d] = W[c*128+p, d] so the load
    # is one fully-contiguous DMA (2KB/partition descriptors)
    wq = nc.declare_dram_parameter("wq", [P, EC * DK], bf16, isOutput=False)
    wk = nc.declare_dram_parameter("wk", [P, EC * DK], bf16, isOutput=False)
    wv = nc.declare_dram_parameter("wv", [P, EC * DV], bf16, isOutput=False)
    mc = nc.declare_dram_parameter("mc", [S, SQ], bf16, isOutput=False)
    out_ot = nc.declare_dram_parameter("out_ot", [DV, SQ], bf16, isOutput=True)
    out_acc = nc.declare_dram_parameter("out_acc", [P, SQ], bf16, isOutput=True)

    with ExitStack() as ctx:
        tc = ctx.enter_context(tile.TileContext(nc))
        const_pool = ctx.enter_context(tc.tile_pool(name="const", bufs=1))
        in_pool = ctx.enter_context(tc.tile_pool(name="inputs", bufs=1))
        proj_sb = ctx.enter_context(tc.tile_pool(name="proj", bufs=1))
        p_pool = ctx.enter_context(tc.tile_pool(name="p", bufs=6))
        stat = ctx.enter_context(tc.tile_pool(name="stat", bufs=1))
        proj_ctx = ctx.enter_context(ExitStack())
        proj_ps = proj_ctx.enter_context(
            tc.tile_pool(name="proj_ps", bufs=8, space="PSUM")
        )

        zeros_w = const_pool.tile([P, 512], bf16)
        nc.vector.memset(zeros_w[:], 0.0)
        acc = stat.tile([P, SQ], bf16)
        nc.vector.memset(acc[:], 0.0)

        # --- input DMAs, all on the sync HWDGE ring: FIFO order gives the
        # projection-critical tensors full HBM bandwidth before the masks.
        w_sb = {}
        for name, w in (("wq", wq), ("wk", wk), ("wv", wv)):
            wt = in_pool.tile([P, EC * DK], bf16, tag=name)
            w_sb[name] = wt

        x_sb = []
        for c in range(EC):
            xc = in_pool.tile([P, S], bf16, tag=f"x{c}")
            x_sb.append(xc)

        # mask in MG grouped tiles [128, KTILES//MG, SQ]:
        # m_sb[g][p, i, q] = mc[(g*KTILES//MG + i)*128 + p, q]
        TPG = KTILES // MG  # key tiles per mask group
        m_sb = []
        for g in range(MG):
            mtile = in_pool.tile([P, TPG, SQ], bf16, tag=f"m{g}")
            m_sb.append(mtile)

        nc.sync.dma_start(w_sb["wq"][:], wq[:, :])
        nc.sync.dma_start(x_sb[0][:], xt[0:P, :])
        nc.sync.dma_start(w_sb["wk"][:], wk[:, :])
        nc.sync.dma_start(w_sb["wv"][:], wv[:, :])
        for c in range(1, EC):
            nc.sync.dma_start(x_sb[c][:], xt[c * P : (c + 1) * P, :])
        mc_v = mc.rearrange("(g i p) q -> g p i q", g=MG, i=TPG, p=P)
        for g in range(2):
            nc.sync.dma_start(m_sb[g][:, :, :], mc_v[g])

        # --- PE warmup: dummy matmuls on zeros while the first DMAs land,
        # so the HAM clock-gate un-throttles (1.2 -> 2.4 GHz) before the
        # real projection matmuls start.
        warm_ps = proj_ps.tile([P, 512], f32, tag="pps")
        for _ in range(N_WARMUP_MM):
            nc.tensor.matmul(
                warm_ps[:], zeros_w[:, 0:P], zeros_w[:], start=True, stop=True
            )

        # --- Q + V projections interleaved per x-chunk: QT [d, q] and
        # VT [d, k] in four 512-col quarters (2 + 4 PSUM banks).
        qt_sb = proj_sb.tile([P, SQ], bf16)
        vt_sb = proj_sb.tile([P, S], bf16)
        v_sb = proj_sb.tile([P, KTILES, DV], bf16)

        def alloc_ps(n, tag="pps"):
            pss = []
            for _ in range(n):
                ps = proj_ps.tile([P, 512], f32, tag=tag)
                pss.append(ps)
            return pss

        qt_ps = alloc_ps(2)
        vt_ps = alloc_ps(4)
        kt0_ps = alloc_ps(2)
        for c in range(EC):
            for j in range(2):
                nc.tensor.matmul(
                    qt_ps[j][:],
                    w_sb["wq"][:, c * DK : (c + 1) * DK],
                    x_sb[c][:, j * 512 : (j + 1) * 512],
                    start=(c == 0),
                    stop=(c == EC - 1),
                )
            for g in range(4):
                nc.tensor.matmul(
                    vt_ps[g][:],
                    w_sb["wv"][:, c * DV : (c + 1) * DV],
                    x_sb[c][:, g * 512 : (g + 1) * 512],
                    start=(c == 0),
                    stop=(c == EC - 1),
                )
            for j in range(2):
                nc.tensor.matmul(
                    kt0_ps[j][:],
                    w_sb["wk"][:, c * DK : (c + 1) * DK],
                    x_sb[c][:, j * 512 : (j + 1) * 512],
                    start=(c == 0),
                    stop=(c == EC - 1),
                )
        # scores_0 gates everything: qt/kt0 copies first on BOTH engines,
        # vt copies after; the V transpose goes on the sync ring (between
        # mask groups m1 and m2) so the ACT queue reaches exp_0 immediately
        kt0_sb = proj_sb.tile([P, SQ], bf16)
        kt1_sb = proj_sb.tile([P, SQ], bf16)
        nc.scalar.copy(qt_sb[:, 0:512], qt_ps[0][:])
        nc.vector.tensor_copy(qt_sb[:, 512:1024], qt_ps[1][:])
        nc.scalar.copy(kt0_sb[:, 0:512], kt0_ps[0][:])
        nc.vector.tensor_copy(kt0_sb[:, 512:1024], kt0_ps[1][:])
        for g in range(4):
            eng = nc.scalar.copy if g % 2 == 0 else nc.vector.tensor_copy
            eng(vt_sb[:, g * 512 : (g + 1) * 512], vt_ps[g][:])
        # ONE batched transpose: v_sb[k, t, dv] = vt_sb[dv, t*128+k]
        nc.sync.dma_start_transpose(v_sb[:, :, :], vt_sb[:, :])
        for g in range(2, MG):
            nc.sync.dma_start(m_sb[g][:, :, :], mc_v[g])
        proj_ctx.close()  # free all 8 projection PSUM banks
        # kt1 accumulates during attention tiles 0..3 (2 banks), st double-
        # buffered (4 banks), ot accumulator (2 banks): exactly 8
        kt1_pool = ctx.enter_context(tc.tile_pool(name="kt1_ps", bufs=2, space="PSUM"))
        kt1_ps = []
        for _ in range(2):
            kp = kt1_pool.tile([P, 512], f32, tag="kt1")
            kt1_ps.append(kp)

        st_pool = ctx.enter_context(tc.tile_pool(name="st_ps", bufs=2, space="PSUM"))
        ot_pool = ctx.enter_context(tc.tile_pool(name="ot_ps", bufs=1, space="PSUM"))
        ot = ot_pool.tile([P, SQ], f32)  # OT [dv, q] accumulator

        # --- attention over key tiles; K half 1 (key tiles 8..15) is
        # interleaved 2 matmuls per tile into tiles 0..7, and AV trails
        # scores by AV_LAG tiles so the V transpose never stalls PE ---
        p_tiles = [None] * KTILES

        def kt_slice(t):
            src = kt0_sb if t < 8 else kt1_sb
            return src[:, (t % 8) * P : (t % 8 + 1) * P]

        def emit_scores(t):
            st = st_pool.tile([P, SQ], f32, tag="st")
            for j in range(2):
                nc.tensor.matmul(
                    st[:, j * 512 : (j + 1) * 512],
                    kt_slice(t),
                    qt_sb[:, j * 512 : (j + 1) * 512],
                    start=True,
                    stop=True,
                )
            p = p_pool.tile([P, SQ], bf16, tag="p")
            p_tiles[t] = p
            nc.scalar.activation(p[:], st[:], mybir.ActivationFunctionType.Exp)
            # zero the masked entries: exp(s - 1e15*m) == exp(s) * (1 - m)
            nc.vector.tensor_mul(p[:], p[:], m_sb[t // (KTILES // MG)][:, t % (KTILES // MG), :])
            # acc += p runs one tile late so AV_t never waits behind it on
            # DVE; the last three adds move past the loop entirely so the
            # final AV matmuls only ever wait on their own mask-multiply
            if 0 < t <= KTILES - 3:
                nc.vector.tensor_add(acc[:], acc[:], p_tiles[t - 1][:])

        def emit_av(t):
            p = p_tiles[t]
            for j in range(2):
                nc.tensor.matmul(
                    ot[:, j * 512 : (j + 1) * 512],
                    v_sb[:, t, :],
                    p[:, j * 512 : (j + 1) * 512],
                    start=(t == 0),
                    stop=(t == KTILES - 1),
                )

        for t in range(KTILES):
            emit_scores(t)
            if t < 4:
                # four K-half-1 matmuls per tile: (j, c) pairs in c-major
                # order so each PSUM group accumulates chunks in sequence
                for i in range(4):
                    idx = 4 * t + i
                    j, c = idx // 8, idx % 8
                    nc.tensor.matmul(
                        kt1_ps[j][:],
                        w_sb["wk"][:, c * DK : (c + 1) * DK],
                        x_sb[c][:, 1024 + j * 512 : 1024 + (j + 1) * 512],
                        start=(c == 0),
                        stop=(c == EC - 1),
                    )
                if t == 3:
                    nc.scalar.copy(kt1_sb[:, 0:512], kt1_ps[0][:])
                    nc.vector.tensor_copy(kt1_sb[:, 512:1024], kt1_ps[1][:])
            if t >= AV_LAG:
                emit_av(t - AV_LAG)
        for t in range(KTILES - AV_LAG, KTILES):
            emit_av(t)
        for t in range(KTILES - 3, KTILES):
            nc.vector.tensor_add(acc[:], acc[:], p_tiles[t][:])

        # --- epilogue: ship OT (unnormalized) and acc; host normalizes ---
        o_sb = stat.tile([P, SQ], bf16)
        nc.scalar.copy(o_sb[:, 0:512], ot[:, 0:512])
        nc.vector.tensor_copy(o_sb[:, 512:1024], ot[:, 512:1024])
        nc.sync.dma_start(out_ot[:, :], o_sb[:])
        nc.scalar.dma_start(out_acc[:, :], acc[:])

    nc.compile()
    return nc


_NC_CACHE = None


def kernel(inputs, mask, Wq, Wk, Wv):
    global _NC_CACHE, LAST_RESULT
    inputs = np.asarray(inputs)
    mask = np.asarray(mask)
    bf = ml_dtypes.bfloat16
    scale = np.float32(1.0 / math.sqrt(DK))

    def pack_w(w):  # [E, DK] -> [p, c*DK+d] = w[c*128+p, d]
        w = np.asarray(w).astype(bf)
        return np.ascontiguousarray(
            w.reshape(EC, P, DK).transpose(1, 0, 2).reshape(P, EC * DK)
        )

    wq_h = pack_w(np.asarray(Wq) * scale)
    wk_h = pack_w(Wk)
    wv_h = pack_w(Wv)

    if _NC_CACHE is None:
        _NC_CACHE = build()
    nc = _NC_CACHE

    in_maps = []
    for core in range(8):
        b, h = divmod(core, 2)
        q0 = h * SQ
        idx = np.r_[q0:S, 0:q0]  # rotate so this core's queries come first
        xb = inputs[b]  # [S, E] f32
        xt_core = np.ascontiguousarray(xb[idx].T).astype(bf)  # [E, S]
        mc_core = np.ascontiguousarray(
            (1 - mask[b, q0 : q0 + SQ, :][:, idx]).T.astype(np.float32)
        ).astype(bf)  # [S, SQ] complement, [key, query]
        in_maps.append(
            {"xt": xt_core, "wq": wq_h, "wk": wk_h, "wv": wv_h, "mc": mc_core}
        )

    res = run_bass_kernel_spmd(nc, in_maps, list(range(8)), trace=TRACE)
    LAST_RESULT = res
    outp = np.empty((B, S, DV), np.float32)
    for core in range(8):
        b, h = divmod(core, 2)
        q0 = h * SQ
        ot = np.asarray(res.results[core]["out_ot"]).astype(np.float32)  # [DV, SQ]
        acc = np.asarray(res.results[core]["out_acc"]).astype(np.float32)  # [P, SQ]
        rowsum = acc.sum(axis=0)  # [SQ]
        outp[b, q0 : q0 + SQ, :] = (ot / rowsum[None, :]).T
    return outp
